# revision 20
# baseline (speedup 1.0000x reference)
"""Trainium2 Bass kernel for the EdgeMask problem.

Computes, for h (B,T,N,d), I_full (B,T,N,N), MLP params W1 (2d,hid) b1 (hid,)
W2 (hid,) b2 (1,):
    li = h @ W1[:d]; lj = h @ W1[d:]
    hid = relu(li[:,:,:,None,:] + lj[:,:,None,:,:] + b1)
    M = sigmoid(hid @ W2 + b2);  I_sparse = I_full * M
Returns (I_sparse, M).

Sharding: data-parallel over B across 8 NeuronCores (B=8), no collectives.

Per-core layout (per (t) slice, N=128 nodes, d=128, K=32 hidden):
  - hT = h[t].T via PE transpose (d on partitions)
  - liT/ljT via col-tiled PE matmuls with W1a/W1b as stationary operands.
    Partition stacking p = 32*gp + k (4 replicas of the 32 hidden units).
    "Group" g covers rows i in {g, g+32, g+64, g+96} (i = g + 32*gp).
      R[32gp+k, j]  = lj[j,k] + b1[k]        (replicated 4x, ACT adds b1)
      S[32gp+k, g]  = li[g+32gp, k]          (li "stack", fp32)
  - Pointwise (the N^2*K hot loop): for each group g one fused op
      hid_g = max(R + S[:,g], 0)   -- DVE tensor_scalar(add,max) / ACT Relu+bias
  - Reduce over k via PE: lhsT = blockdiag(W2 x4) (128,4), col-tiled 4-way,
    rhs = 4 groups' hid packed into (128,512):
      logits'[i=4w+c+32m, j] at PSUM[32q+m, 128c+j]  (w = 4*half + q)
  - Compact 2 PSUM banks -> dense (128,128) via DMA, sigmoid(+b2) on ACT,
    I_full * M on DVE, DMA out.
"""

import functools

import numpy as np

import concourse.bass as bass
import concourse.mybir as mybir
import concourse.tile as tile
from concourse import bacc

F32 = mybir.dt.float32
F16 = mybir.dt.float16

B = 8
T = 32
N = 128
D = 128
K = 32  # hidden
NCORES = 8

AFT = mybir.ActivationFunctionType
ALU = mybir.AluOpType

# dtype of the hid (pointwise+reduce) path: F16 -> DVE 4x mode, F32 exact
HID_DT = F16
HID_NP = np.float16 if HID_DT == F16 else np.float32

# pointwise split: first ACT_SHARE_G groups on ACT, last POOL_SHARE_G on
# GPSIMD, rest on DVE
ACT_SHARE_G = 3
POOL_SHARE_G = 10
HID_BUFS = 24


def _build(t_slices: int = T, skip=()):
    """skip: subset of {"pointwise","reduce","sigmoid","bounce","mask","lilj"} (debug)"""
    nc = bacc.Bacc(
        "TRN2", target_bir_lowering=False, debug=False, num_devices=NCORES
    )

    ht_d = nc.dram_tensor("ht", [D, t_slices * N], HID_DT, kind="ExternalInput")
    i_d = nc.dram_tensor("ifull", [t_slices, N, N], F32, kind="ExternalInput")
    w1a_d = nc.dram_tensor("w1a", [D, K], HID_DT, kind="ExternalInput")
    w1b_d = nc.dram_tensor("w1b", [D, K], HID_DT, kind="ExternalInput")
    b1t_d = nc.dram_tensor("b1t", [128, 1], F32, kind="ExternalInput")
    wd_d = nc.dram_tensor("wd", [128, 32], HID_DT, kind="ExternalInput")
    b2t_d = nc.dram_tensor("b2t", [128, 1], F32, kind="ExternalInput")
    perm_d = nc.dram_tensor("perm", [128, 8 * 128], HID_DT, kind="ExternalInput")

    isp_d = nc.dram_tensor("isp", [t_slices, N, N], F32, kind="ExternalOutput")
    m_d = nc.dram_tensor("m", [t_slices, N, N], F32, kind="ExternalOutput")

    with tile.TileContext(nc) as tc:
        with (
            tc.tile_pool(name="const", bufs=1) as cpool,
            tc.tile_pool(name="hin", bufs=4) as hpool,
            tc.tile_pool(name="hts", bufs=3) as htpool,
            tc.tile_pool(name="rs", bufs=3) as rspool,
            tc.tile_pool(name="hid", bufs=HID_BUFS) as hidpool,
            tc.tile_pool(name="io", bufs=4) as iopool,
            tc.tile_pool(name="outp", bufs=3) as opool,
            tc.tile_pool(name="psum", bufs=2, space="PSUM") as ppool,
        ):
            w1a_sb = cpool.tile([D, K], HID_DT)
            nc.sync.dma_start(w1a_sb[:], w1a_d[:])
            w1b_sb = cpool.tile([D, K], HID_DT)
            nc.sync.dma_start(w1b_sb[:], w1b_d[:])
            b1t_sb = cpool.tile([128, 1], F32)
            nc.sync.dma_start(b1t_sb[:], b1t_d[:])
            wd_sb = cpool.tile([128, 32], HID_DT)
            nc.sync.dma_start(wd_sb[:], wd_d[:])
            b2t_sb = cpool.tile([128, 1], F32)
            nc.sync.dma_start(b2t_sb[:], b2t_d[:])
            perm_sb = cpool.tile([128, 8 * 128], HID_DT)
            nc.sync.dma_start(perm_sb[:], perm_d[:])
            # all slices' hT in one DMA (big contiguous runs)
            htall_sb = cpool.tile([D, t_slices * N], HID_DT)
            nc.sync.dma_start(htall_sb[:], ht_d[:])

            for t in range(t_slices):
                ht_sb = htall_sb[:, t * N : (t + 1) * N]

                # ---- liT / ljT, col-tiled (4 concurrent 32-col groups) ----
                lilj_ps = ppool.tile([128, N + K], F32, tag="lilj")
                for gp in range(4):
                    # ljT replicated: out[32gp+k, j] = lj[j, k]
                    nc.tensor.matmul(
                        lilj_ps[32 * gp : 32 * gp + 32, 0:N],
                        w1b_sb[:],
                        ht_sb,
                        tile_position=(0, 32 * gp),
                    )
                for gp in range(4):
                    # li stack: out[32gp+k, g] = li[g+32gp, k]
                    nc.tensor.matmul(
                        lilj_ps[32 * gp : 32 * gp + 32, N : N + K],
                        w1a_sb[:],
                        ht_sb[:, 32 * gp : 32 * gp + 32],
                        tile_position=(0, 32 * gp),
                    )

                # R = ljT_rep + b1 (cast to HID_DT); S = li stack (fp32)
                r_sb = rspool.tile([128, N], HID_DT, tag="r")
                nc.scalar.activation(
                    r_sb[:], lilj_ps[:, 0:N], AFT.Identity, bias=b1t_sb[:, 0:1]
                )
                s_sb = rspool.tile([128, K], F32, tag="s")
                nc.vector.tensor_copy(s_sb[:], lilj_ps[:, N : N + K])

                # ---- pointwise: hid_g = relu(R + S[:, g]) ----
                hbufs = [
                    hidpool.tile([128, 4 * N], HID_DT, tag="hid", name=f"hb{w}")
                    for w in range(8)
                ]
                for g in range(K if "pointwise" not in skip else 0):
                    w, c = divmod(g, 4)
                    dst = hbufs[w][:, c * N : (c + 1) * N]
                    if g < ACT_SHARE_G:
                        nc.scalar.activation(
                            dst, r_sb[:], AFT.Relu, bias=s_sb[:, g : g + 1]
                        )
                    elif g >= K - POOL_SHARE_G:
                        nc.gpsimd.tensor_scalar(
                            dst, r_sb[:], s_sb[:, g : g + 1], 0.0, ALU.add, ALU.max
                        )
                    else:
                        nc.vector.tensor_scalar(
                            dst, r_sb[:], s_sb[:, g : g + 1], 0.0, ALU.add, ALU.max
                        )

                # ---- reduce over k on PE (col-tiled, 2 waves of 4) ----
                l_ps = [
                    ppool.tile([128, 4 * N], F32, tag="l0", name="l0"),
                    ppool.tile([128, 4 * N], F32, tag="l1", name="l1"),
                ]
                for w in range(8 if "reduce" not in skip else 0):
                    half, q = divmod(w, 4)
                    nc.tensor.matmul(
                        l_ps[half][32 * q : 32 * q + 32, :],
                        wd_sb[:],
                        hbufs[w][:],
                        tile_position=(0, 32 * q),
                    )

                # ---- sigmoid directly on (sparse) PSUM: the PSUM exit ----
                # used rows are {32q+m : q,m<4} subset of [0,100)
                msp = [
                    opool.tile([128, 4 * N], HID_DT, tag="msp0", name="msp0"),
                    opool.tile([128, 4 * N], HID_DT, tag="msp1", name="msp1"),
                ]
                for half in range(2 if "sigmoid" not in skip else 0):
                    nc.scalar.activation(
                        msp[half][:],
                        l_ps[half][:],
                        AFT.Sigmoid,
                        bias=b2t_sb[:, 0:1],
                    )

                # ---- un-permute M on PE: 8 accumulating matmuls with 0/1
                # permutation matrices; P_b[32q+m, 32m+16h+4q+c] = 1, b=4h+c
                mp_ps = ppool.tile([128, N], F32, tag="mp")
                for b in range(8):
                    h, c = divmod(b, 4)
                    nc.tensor.matmul(
                        mp_ps[:],
                        perm_sb[:, 128 * b : 128 * (b + 1)],
                        msp[h][:, 128 * c : 128 * (c + 1)],
                        start=(b == 0),
                        stop=(b == 7),
                    )
                m_sb = opool.tile([128, N], F32, tag="m")
                nc.vector.tensor_copy(m_sb[:], mp_ps[:])
                i_sb = iopool.tile([N, N], F32, tag="i")
                nc.sync.dma_start(i_sb[:], i_d[t, :, :])
                isp_sb = opool.tile([N, N], F32, tag="isp")
                nc.vector.tensor_tensor(isp_sb[:], i_sb[:], m_sb[:], ALU.mult)
                nc.sync.dma_start(m_d[t, :, :], m_sb[:])
                nc.sync.dma_start(isp_d[t, :, :], isp_sb[:])

    nc.compile()
    return nc


def make_aux_inputs(W1, b1, W2, b2):
    W1 = np.asarray(W1)
    w1a = np.ascontiguousarray(W1[:D]).astype(HID_NP)
    w1b = np.ascontiguousarray(W1[D:]).astype(HID_NP)
    b1t = np.ascontiguousarray(np.tile(np.asarray(b1, np.float32), 4).reshape(128, 1))
    # col m carries W2 at partition-block (m % 4): every PSUM output row of the
    # reduce matmul is then a valid (replicated) logits row
    wd = np.zeros((128, 32), HID_NP)
    for m in range(32):
        gp = m % 4
        wd[32 * gp : 32 * gp + 32, m] = np.asarray(W2)
    b2t = np.full((128, 1), np.asarray(b2, np.float32)[0], np.float32)
    perm = np.zeros((8, 128, 128), np.float32)
    for h in range(2):
        for c in range(4):
            for q in range(4):
                for m in range(4):
                    perm[4 * h + c, 32 * q + m, 32 * m + 16 * h + 4 * q + c] = 1.0
    perm = np.ascontiguousarray(np.concatenate(list(perm), axis=1)).astype(HID_NP)
    return {
        "perm": perm,
        "w1a": w1a,
        "w1b": w1b,
        "b1t": b1t,
        "wd": wd,
        "b2t": b2t,
    }


TRACE = False
LAST_RESULTS = None


@functools.lru_cache(maxsize=1)
def _built_nc():
    return _build(T)


def kernel(**inputs):
    from concourse.bass_utils import run_bass_kernel_spmd

    h = np.asarray(inputs["h"])
    # (B, T, N, D) -> (B, D, T*N) so one DMA per core loads all hT with 8KB runs
    ht = np.ascontiguousarray(np.transpose(h, (0, 3, 1, 2)).reshape(B, D, -1)).astype(
        HID_NP
    )
    ifull = np.ascontiguousarray(np.asarray(inputs["I_full"], np.float32))
    aux = make_aux_inputs(inputs["W1"], inputs["b1"], inputs["W2"], inputs["b2"])

    nc = _built_nc()
    in_maps = [{"ht": ht[c], "ifull": ifull[c], **aux} for c in range(NCORES)]
    res = run_bass_kernel_spmd(
        nc, in_maps, core_ids=list(range(NCORES)), trace=TRACE
    )
    global LAST_RESULTS
    LAST_RESULTS = res
    isp = np.stack([res.results[c]["isp"] for c in range(NCORES)])
    m = np.stack([res.results[c]["m"] for c in range(NCORES)])
    return isp, m


# revision 27
# speedup vs baseline: 1.0717x; 1.0717x over previous
"""Trainium2 Bass kernel for the EdgeMask problem.

Computes, for h (B,T,N,d), I_full (B,T,N,N), MLP params W1 (2d,hid) b1 (hid,)
W2 (hid,) b2 (1,):
    li = h @ W1[:d]; lj = h @ W1[d:]
    hid = relu(li[:,:,:,None,:] + lj[:,:,None,:,:] + b1)
    M = sigmoid(hid @ W2 + b2);  I_sparse = I_full * M
Returns (I_sparse, M).

Sharding: data-parallel over B across 8 NeuronCores (B=8), no collectives.

Per-core layout (per (t) slice, N=128 nodes, d=128, K=32 hidden):
  - hT = h[t].T via PE transpose (d on partitions)
  - liT/ljT via col-tiled PE matmuls with W1a/W1b as stationary operands.
    Partition stacking p = 32*gp + k (4 replicas of the 32 hidden units).
    "Group" g covers rows i in {g, g+32, g+64, g+96} (i = g + 32*gp).
      R[32gp+k, j]  = lj[j,k] + b1[k]        (replicated 4x, ACT adds b1)
      S[32gp+k, g]  = li[g+32gp, k]          (li "stack", fp32)
  - Pointwise (the N^2*K hot loop): for each group g one fused op
      hid_g = max(R + S[:,g], 0)   -- DVE tensor_scalar(add,max) / ACT Relu+bias
  - Reduce over k via PE: lhsT = blockdiag(W2 x4) (128,4), col-tiled 4-way,
    rhs = 4 groups' hid packed into (128,512):
      logits'[i=4w+c+32m, j] at PSUM[32q+m, 128c+j]  (w = 4*half + q)
  - Compact 2 PSUM banks -> dense (128,128) via DMA, sigmoid(+b2) on ACT,
    I_full * M on DVE, DMA out.
"""

import functools

import numpy as np

import concourse.bass as bass
import concourse.mybir as mybir
import concourse.tile as tile
from concourse import bacc

F32 = mybir.dt.float32
F16 = mybir.dt.float16

B = 8
T = 32
N = 128
D = 128
K = 32  # hidden
NCORES = 8

AFT = mybir.ActivationFunctionType
ALU = mybir.AluOpType

# dtype of the hid (pointwise+reduce) path: F16 -> DVE 4x mode, F32 exact
HID_DT = F16
HID_NP = np.float16 if HID_DT == F16 else np.float32

# pointwise split: first ACT_SHARE_G groups on ACT, last POOL_SHARE_G on
# GPSIMD, rest on DVE
ACT_SHARE_G = 3
POOL_SHARE_G = 10
HID_BUFS = 24
SIG_DENSE = False
PW_SPREAD = False
MASK_ON_POOL = False
IO_BUFS = 4
OUT_BUFS = 3
RS_BUFS = 3
LILJ_BUFS = 2
MP_BUFS = 2
S_ON_ACT = False


def _pw_engine(g):
    if PW_SPREAD:
        # interleave: pool every 3rd, act sprinkled, rest dve
        w, c = divmod(g, 4)
        if c == 3 and w >= 8 - POOL_SHARE_G // 4 * 4:
            pass
        seq = (["dve"] * (K - ACT_SHARE_G - POOL_SHARE_G)
               + ["pool"] * POOL_SHARE_G + ["act"] * ACT_SHARE_G)
        # round-robin-ish deterministic shuffle
        return seq[(g * 7) % K]
    if g < ACT_SHARE_G:
        return "act"
    if g >= K - POOL_SHARE_G:
        return "pool"
    return "dve"


def _build(t_slices: int = T, skip=()):
    """skip: subset of {"pointwise","reduce","sigmoid","bounce","mask","lilj"} (debug)"""
    nc = bacc.Bacc(
        "TRN2", target_bir_lowering=False, debug=False, num_devices=NCORES
    )

    ht_d = nc.dram_tensor("ht", [D, t_slices * N], HID_DT, kind="ExternalInput")
    i_d = nc.dram_tensor("ifull", [t_slices, N, N], F32, kind="ExternalInput")
    w1a_d = nc.dram_tensor("w1a", [D, K], HID_DT, kind="ExternalInput")
    w1b_d = nc.dram_tensor("w1b", [D, K], HID_DT, kind="ExternalInput")
    b1t_d = nc.dram_tensor("b1t", [128, 1], F32, kind="ExternalInput")
    wd_d = nc.dram_tensor("wd", [128, 32], HID_DT, kind="ExternalInput")
    b2t_d = nc.dram_tensor("b2t", [128, 1], F32, kind="ExternalInput")
    perm_d = nc.dram_tensor("perm", [128, 8 * 128], HID_DT, kind="ExternalInput")

    # merged output: [..., 0:N] = M, [..., N:2N] = I_sparse (one store per slice)
    mi_d = nc.dram_tensor("mi", [t_slices, N, 2 * N], F32, kind="ExternalOutput")

    with tile.TileContext(nc) as tc:
        with (
            tc.tile_pool(name="const", bufs=1) as cpool,
            tc.tile_pool(name="hin", bufs=4) as hpool,
            tc.tile_pool(name="hts", bufs=3) as htpool,
            tc.tile_pool(name="rs", bufs=RS_BUFS) as rspool,
            tc.tile_pool(name="hid", bufs=HID_BUFS) as hidpool,
            tc.tile_pool(name="io", bufs=IO_BUFS) as iopool,
            tc.tile_pool(name="outp", bufs=OUT_BUFS) as opool,
            tc.tile_pool(name="psum", bufs=2, space="PSUM") as ppool,
        ):
            w1a_sb = cpool.tile([D, K], HID_DT)
            nc.sync.dma_start(w1a_sb[:], w1a_d[:])
            w1b_sb = cpool.tile([D, K], HID_DT)
            nc.sync.dma_start(w1b_sb[:], w1b_d[:])
            b1t_sb = cpool.tile([128, 1], F32)
            nc.sync.dma_start(b1t_sb[:], b1t_d[:])
            wd_sb = cpool.tile([128, 32], HID_DT)
            nc.sync.dma_start(wd_sb[:], wd_d[:])
            b2t_sb = cpool.tile([128, 1], F32)
            nc.sync.dma_start(b2t_sb[:], b2t_d[:])
            perm_sb = cpool.tile([128, 8 * 128], HID_DT)
            nc.sync.dma_start(perm_sb[:], perm_d[:])
            # all slices' hT in one DMA (big contiguous runs)
            htall_sb = cpool.tile([D, t_slices * N], HID_DT)
            nc.sync.dma_start(htall_sb[:], ht_d[:])

            for t in range(t_slices):
                ht_sb = htall_sb[:, t * N : (t + 1) * N]

                # ---- liT / ljT, col-tiled (4 concurrent 32-col groups) ----
                lilj_ps = ppool.tile([128, N + K], F32, tag="lilj", bufs=LILJ_BUFS)
                for gp in range(4):
                    # ljT replicated: out[32gp+k, j] = lj[j, k]
                    nc.tensor.matmul(
                        lilj_ps[32 * gp : 32 * gp + 32, 0:N],
                        w1b_sb[:],
                        ht_sb,
                        tile_position=(0, 32 * gp),
                        skip_group_check=True,
                    )
                for gp in range(4):
                    # li stack: out[32gp+k, g] = li[g+32gp, k]
                    nc.tensor.matmul(
                        lilj_ps[32 * gp : 32 * gp + 32, N : N + K],
                        w1a_sb[:],
                        ht_sb[:, 32 * gp : 32 * gp + 32],
                        tile_position=(0, 32 * gp),
                        skip_group_check=True,
                    )

                # R = ljT_rep + b1 (cast to HID_DT); S = li stack (fp32)
                r_sb = rspool.tile([128, N], HID_DT, tag="r")
                nc.scalar.activation(
                    r_sb[:], lilj_ps[:, 0:N], AFT.Identity, bias=b1t_sb[:, 0:1]
                )
                s_sb = rspool.tile([128, K], F32, tag="s")
                if S_ON_ACT:
                    nc.scalar.copy(s_sb[:], lilj_ps[:, N : N + K])
                else:
                    nc.vector.tensor_copy(s_sb[:], lilj_ps[:, N : N + K])

                # ---- pointwise: hid_g = relu(R + S[:, g]) ----
                hbufs = [
                    hidpool.tile([128, 4 * N], HID_DT, tag="hid", name=f"hb{w}")
                    for w in range(8)
                ]
                for g in range(K if "pointwise" not in skip else 0):
                    w, c = divmod(g, 4)
                    dst = hbufs[w][:, c * N : (c + 1) * N]
                    eng = _pw_engine(g)
                    if eng == "act":
                        nc.scalar.activation(
                            dst, r_sb[:], AFT.Relu, bias=s_sb[:, g : g + 1]
                        )
                    elif eng == "pool":
                        nc.gpsimd.tensor_scalar(
                            dst, r_sb[:], s_sb[:, g : g + 1], 0.0, ALU.add, ALU.max
                        )
                    else:
                        nc.vector.tensor_scalar(
                            dst, r_sb[:], s_sb[:, g : g + 1], 0.0, ALU.add, ALU.max
                        )

                # ---- reduce over k on PE (col-tiled, 2 waves of 4) ----
                l_ps = [
                    ppool.tile([128, 4 * N], F32, tag="l0", name="l0"),
                    ppool.tile([128, 4 * N], F32, tag="l1", name="l1"),
                ]
                for w in range(8 if "reduce" not in skip else 0):
                    half, q = divmod(w, 4)
                    nc.tensor.matmul(
                        l_ps[half][32 * q : 32 * q + 32, :],
                        wd_sb[:],
                        hbufs[w][:],
                        tile_position=(0, 32 * q),
                    )

                # ---- PSUM exits (cast fp16) to SBUF ----
                # SIG_DENSE: raw-logit copies here, sigmoid after the permute.
                # else: sigmoid(+b2) applied here (sparse), permute carries M.
                lsp = [
                    opool.tile([128, 4 * N], HID_DT, tag="lsp0", name="lsp0"),
                    opool.tile([128, 4 * N], HID_DT, tag="lsp1", name="lsp1"),
                ]
                if SIG_DENSE:
                    nc.scalar.copy(lsp[0][:], l_ps[0][:])
                    nc.vector.tensor_copy(lsp[1][:], l_ps[1][:])
                else:
                    for half in range(2):
                        nc.scalar.activation(
                            lsp[half][:], l_ps[half][:], AFT.Sigmoid,
                            bias=b2t_sb[:, 0:1],
                        )

                # ---- un-permute logits on PE: 8 accumulating matmuls with 0/1
                # permutation matrices; P_b[32q+m, 32m+16h+4q+c] = 1, b=4h+c
                mp_ps = ppool.tile([128, N], F32, tag="mp", bufs=MP_BUFS)
                for b in range(8):
                    h, c = divmod(b, 4)
                    nc.tensor.matmul(
                        mp_ps[:],
                        perm_sb[:, 128 * b : 128 * (b + 1)],
                        lsp[h][:, 128 * c : 128 * (c + 1)],
                        start=(b == 0),
                        stop=(b == 7),
                    )
                # dense-PSUM exit; M lands in mi[:, 0:N]
                mi_sb = opool.tile([128, 2 * N], F32, tag="mi")
                if SIG_DENSE:
                    nc.scalar.activation(
                        mi_sb[:, 0:N], mp_ps[:], AFT.Sigmoid, bias=b2t_sb[:, 0:1]
                    )
                else:
                    nc.vector.tensor_copy(mi_sb[:, 0:N], mp_ps[:])
                i_sb = iopool.tile([N, N], F32, tag="i")
                nc.sync.dma_start(i_sb[:], i_d[t, :, :])
                if MASK_ON_POOL:
                    nc.gpsimd.tensor_tensor(
                        mi_sb[:, N : 2 * N], i_sb[:], mi_sb[:, 0:N], ALU.mult
                    )
                else:
                    nc.vector.tensor_tensor(
                        mi_sb[:, N : 2 * N], i_sb[:], mi_sb[:, 0:N], ALU.mult
                    )
                nc.sync.dma_start(mi_d[t, :, :], mi_sb[:])

    nc.compile()
    return nc


def make_aux_inputs(W1, b1, W2, b2):
    W1 = np.asarray(W1)
    w1a = np.ascontiguousarray(W1[:D]).astype(HID_NP)
    w1b = np.ascontiguousarray(W1[D:]).astype(HID_NP)
    b1t = np.ascontiguousarray(np.tile(np.asarray(b1, np.float32), 4).reshape(128, 1))
    # col m carries W2 at partition-block (m % 4): every PSUM output row of the
    # reduce matmul is then a valid (replicated) logits row
    wd = np.zeros((128, 32), HID_NP)
    for m in range(32):
        gp = m % 4
        wd[32 * gp : 32 * gp + 32, m] = np.asarray(W2)
    b2t = np.full((128, 1), np.asarray(b2, np.float32)[0], np.float32)
    perm = np.zeros((8, 128, 128), np.float32)
    for h in range(2):
        for c in range(4):
            for q in range(4):
                for m in range(4):
                    perm[4 * h + c, 32 * q + m, 32 * m + 16 * h + 4 * q + c] = 1.0
    perm = np.ascontiguousarray(np.concatenate(list(perm), axis=1)).astype(HID_NP)
    return {
        "perm": perm,
        "w1a": w1a,
        "w1b": w1b,
        "b1t": b1t,
        "wd": wd,
        "b2t": b2t,
    }


TRACE = False
LAST_RESULTS = None


@functools.lru_cache(maxsize=1)
def _built_nc():
    return _build(T)


def kernel(**inputs):
    from concourse.bass_utils import run_bass_kernel_spmd

    h = np.asarray(inputs["h"])
    # (B, T, N, D) -> (B, D, T*N) so one DMA per core loads all hT with 8KB runs
    ht = np.ascontiguousarray(np.transpose(h, (0, 3, 1, 2)).reshape(B, D, -1)).astype(
        HID_NP
    )
    ifull = np.ascontiguousarray(np.asarray(inputs["I_full"], np.float32))
    aux = make_aux_inputs(inputs["W1"], inputs["b1"], inputs["W2"], inputs["b2"])

    nc = _built_nc()
    in_maps = [{"ht": ht[c], "ifull": ifull[c], **aux} for c in range(NCORES)]
    res = run_bass_kernel_spmd(
        nc, in_maps, core_ids=list(range(NCORES)), trace=TRACE
    )
    global LAST_RESULTS
    LAST_RESULTS = res
    mi = np.stack([res.results[c]["mi"] for c in range(NCORES)])
    return np.ascontiguousarray(mi[..., N:]), np.ascontiguousarray(mi[..., :N])


# revision 28
# speedup vs baseline: 17661.4288x; 16479.7733x over previous
"""Trainium2 Bass kernel for the EdgeMask problem.

Computes, for h (B,T,N,d), I_full (B,T,N,N), MLP params W1 (2d,hid) b1 (hid,)
W2 (hid,) b2 (1,):
    li = h @ W1[:d]; lj = h @ W1[d:]
    hid = relu(li[:,:,:,None,:] + lj[:,:,None,:,:] + b1)
    M = sigmoid(hid @ W2 + b2);  I_sparse = I_full * M
Returns (I_sparse, M).

Sharding: data-parallel over B across 8 NeuronCores (B=8), no collectives.

Per-core layout (per (t) slice, N=128 nodes, d=128, K=32 hidden):
  - hT = h[t].T via PE transpose (d on partitions)
  - liT/ljT via col-tiled PE matmuls with W1a/W1b as stationary operands.
    Partition stacking p = 32*gp + k (4 replicas of the 32 hidden units).
    "Group" g covers rows i in {g, g+32, g+64, g+96} (i = g + 32*gp).
      R[32gp+k, j]  = lj[j,k] + b1[k]        (replicated 4x, ACT adds b1)
      S[32gp+k, g]  = li[g+32gp, k]          (li "stack", fp32)
  - Pointwise (the N^2*K hot loop): for each group g one fused op
      hid_g = max(R + S[:,g], 0)   -- DVE tensor_scalar(add,max) / ACT Relu+bias
  - Reduce over k via PE: lhsT = blockdiag(W2 x4) (128,4), col-tiled 4-way,
    rhs = 4 groups' hid packed into (128,512):
      logits'[i=4w+c+32m, j] at PSUM[32q+m, 128c+j]  (w = 4*half + q)
  - Compact 2 PSUM banks -> dense (128,128) via DMA, sigmoid(+b2) on ACT,
    I_full * M on DVE, DMA out.
"""

import functools

import numpy as np

import concourse.bass as bass
import concourse.mybir as mybir
import concourse.tile as tile
from concourse import bacc

F32 = mybir.dt.float32
F16 = mybir.dt.float16

B = 8
T = 32
N = 128
D = 128
K = 32  # hidden
NCORES = 8

AFT = mybir.ActivationFunctionType
ALU = mybir.AluOpType

# dtype of the hid (pointwise+reduce) path: F16 -> DVE 4x mode, F32 exact
HID_DT = F16
HID_NP = np.float16 if HID_DT == F16 else np.float32

# pointwise split: first ACT_SHARE_G groups on ACT, last POOL_SHARE_G on
# GPSIMD, rest on DVE
ACT_SHARE_G = 3
POOL_SHARE_G = 10
HID_BUFS = 24
SIG_DENSE = False
PW_SPREAD = False
MASK_ON_POOL = False
IO_BUFS = 4
OUT_BUFS = 3
RS_BUFS = 3
LILJ_BUFS = 2
MP_BUFS = 2
S_ON_ACT = False


def _pw_engine(g):
    if PW_SPREAD:
        # interleave: pool every 3rd, act sprinkled, rest dve
        w, c = divmod(g, 4)
        if c == 3 and w >= 8 - POOL_SHARE_G // 4 * 4:
            pass
        seq = (["dve"] * (K - ACT_SHARE_G - POOL_SHARE_G)
               + ["pool"] * POOL_SHARE_G + ["act"] * ACT_SHARE_G)
        # round-robin-ish deterministic shuffle
        return seq[(g * 7) % K]
    if g < ACT_SHARE_G:
        return "act"
    if g >= K - POOL_SHARE_G:
        return "pool"
    return "dve"


def _build(t_slices: int = T, skip=()):
    nc = bacc.Bacc(
        "TRN2", target_bir_lowering=False, debug=False, num_devices=NCORES
    )

    ht_d = nc.dram_tensor("ht", [D, t_slices * N], HID_DT, kind="ExternalInput")
    i_d = nc.dram_tensor("ifull", [t_slices, N, N], F32, kind="ExternalInput")
    w1a_d = nc.dram_tensor("w1a", [D, K], HID_DT, kind="ExternalInput")
    w1b_d = nc.dram_tensor("w1b", [D, K], HID_DT, kind="ExternalInput")
    b1t_d = nc.dram_tensor("b1t", [128, 1], F32, kind="ExternalInput")
    wd_d = nc.dram_tensor("wd", [128, 32], HID_DT, kind="ExternalInput")
    b2t_d = nc.dram_tensor("b2t", [128, 1], F32, kind="ExternalInput")
    perm_d = nc.dram_tensor("perm", [128, 8 * 128], HID_DT, kind="ExternalInput")

    # merged output: [..., 0:N] = M, [..., N:2N] = I_sparse (one store per slice)
    mi_d = nc.dram_tensor("mi", [t_slices, N, 2 * N], F32, kind="ExternalOutput")

    with tile.TileContext(nc) as tc:
        with (
            tc.tile_pool(name="const", bufs=1) as cpool,
            tc.tile_pool(name="hin", bufs=4) as hpool,
            tc.tile_pool(name="hts", bufs=3) as htpool,
            tc.tile_pool(name="rs", bufs=RS_BUFS) as rspool,
            tc.tile_pool(name="hid", bufs=HID_BUFS) as hidpool,
            tc.tile_pool(name="io", bufs=IO_BUFS) as iopool,
            tc.tile_pool(name="outp", bufs=OUT_BUFS) as opool,
            tc.tile_pool(name="psum", bufs=2, space="PSUM") as ppool,
        ):
            w1a_sb = cpool.tile([D, K], HID_DT)
            nc.sync.dma_start(w1a_sb[:], w1a_d[:])
            w1b_sb = cpool.tile([D, K], HID_DT)
            nc.sync.dma_start(w1b_sb[:], w1b_d[:])
            b1t_sb = cpool.tile([128, 1], F32)
            nc.sync.dma_start(b1t_sb[:], b1t_d[:])
            wd_sb = cpool.tile([128, 32], HID_DT)
            nc.sync.dma_start(wd_sb[:], wd_d[:])
            b2t_sb = cpool.tile([128, 1], F32)
            nc.sync.dma_start(b2t_sb[:], b2t_d[:])
            perm_sb = cpool.tile([128, 8 * 128], HID_DT)
            nc.sync.dma_start(perm_sb[:], perm_d[:])
            # all slices' hT in one DMA (big contiguous runs)
            htall_sb = cpool.tile([D, t_slices * N], HID_DT)
            nc.sync.dma_start(htall_sb[:], ht_d[:])

            for t in range(t_slices):
                ht_sb = htall_sb[:, t * N : (t + 1) * N]

                # ---- liT / ljT, col-tiled (4 concurrent 32-col groups) ----
                lilj_ps = ppool.tile([128, N + K], F32, tag="lilj", bufs=LILJ_BUFS)
                for gp in range(4):
                    # ljT replicated: out[32gp+k, j] = lj[j, k]
                    nc.tensor.matmul(
                        lilj_ps[32 * gp : 32 * gp + 32, 0:N],
                        w1b_sb[:],
                        ht_sb,
                        tile_position=(0, 32 * gp),
                        skip_group_check=True,
                    )
                for gp in range(4):
                    # li stack: out[32gp+k, g] = li[g+32gp, k]
                    nc.tensor.matmul(
                        lilj_ps[32 * gp : 32 * gp + 32, N : N + K],
                        w1a_sb[:],
                        ht_sb[:, 32 * gp : 32 * gp + 32],
                        tile_position=(0, 32 * gp),
                        skip_group_check=True,
                    )

                # R = ljT_rep + b1 (cast to HID_DT); S = li stack (fp32)
                r_sb = rspool.tile([128, N], HID_DT, tag="r")
                nc.scalar.activation(
                    r_sb[:], lilj_ps[:, 0:N], AFT.Identity, bias=b1t_sb[:, 0:1]
                )
                s_sb = rspool.tile([128, K], F32, tag="s")
                if S_ON_ACT:
                    nc.scalar.copy(s_sb[:], lilj_ps[:, N : N + K])
                else:
                    nc.vector.tensor_copy(s_sb[:], lilj_ps[:, N : N + K])

                # ---- pointwise: hid_g = relu(R + S[:, g]) ----
                hbufs = [
                    hidpool.tile([128, 4 * N], HID_DT, tag="hid", name=f"hb{w}")
                    for w in range(8)
                ]
                for g in range(K):
                    w, c = divmod(g, 4)
                    dst = hbufs[w][:, c * N : (c + 1) * N]
                    eng = _pw_engine(g)
                    if eng == "act":
                        nc.scalar.activation(
                            dst, r_sb[:], AFT.Relu, bias=s_sb[:, g : g + 1]
                        )
                    elif eng == "pool":
                        nc.gpsimd.tensor_scalar(
                            dst, r_sb[:], s_sb[:, g : g + 1], 0.0, ALU.add, ALU.max
                        )
                    else:
                        nc.vector.tensor_scalar(
                            dst, r_sb[:], s_sb[:, g : g + 1], 0.0, ALU.add, ALU.max
                        )

                # ---- reduce over k on PE (col-tiled, 2 waves of 4) ----
                l_ps = [
                    ppool.tile([128, 4 * N], F32, tag="l0", name="l0"),
                    ppool.tile([128, 4 * N], F32, tag="l1", name="l1"),
                ]
                for w in range(8):
                    half, q = divmod(w, 4)
                    nc.tensor.matmul(
                        l_ps[half][32 * q : 32 * q + 32, :],
                        wd_sb[:],
                        hbufs[w][:],
                        tile_position=(0, 32 * q),
                    )

                # ---- PSUM exits (cast fp16) to SBUF ----
                # SIG_DENSE: raw-logit copies here, sigmoid after the permute.
                # else: sigmoid(+b2) applied here (sparse), permute carries M.
                lsp = [
                    opool.tile([128, 4 * N], HID_DT, tag="lsp0", name="lsp0"),
                    opool.tile([128, 4 * N], HID_DT, tag="lsp1", name="lsp1"),
                ]
                if SIG_DENSE:
                    nc.scalar.copy(lsp[0][:], l_ps[0][:])
                    nc.vector.tensor_copy(lsp[1][:], l_ps[1][:])
                else:
                    for half in range(2):
                        nc.scalar.activation(
                            lsp[half][:], l_ps[half][:], AFT.Sigmoid,
                            bias=b2t_sb[:, 0:1],
                        )

                # ---- un-permute logits on PE: 8 accumulating matmuls with 0/1
                # permutation matrices; P_b[32q+m, 32m+16h+4q+c] = 1, b=4h+c
                mp_ps = ppool.tile([128, N], F32, tag="mp", bufs=MP_BUFS)
                for b in range(8):
                    h, c = divmod(b, 4)
                    nc.tensor.matmul(
                        mp_ps[:],
                        perm_sb[:, 128 * b : 128 * (b + 1)],
                        lsp[h][:, 128 * c : 128 * (c + 1)],
                        start=(b == 0),
                        stop=(b == 7),
                    )
                # dense-PSUM exit; M lands in mi[:, 0:N]
                mi_sb = opool.tile([128, 2 * N], F32, tag="mi")
                if SIG_DENSE:
                    nc.scalar.activation(
                        mi_sb[:, 0:N], mp_ps[:], AFT.Sigmoid, bias=b2t_sb[:, 0:1]
                    )
                else:
                    nc.vector.tensor_copy(mi_sb[:, 0:N], mp_ps[:])
                i_sb = iopool.tile([N, N], F32, tag="i")
                nc.sync.dma_start(i_sb[:], i_d[t, :, :])
                if MASK_ON_POOL:
                    nc.gpsimd.tensor_tensor(
                        mi_sb[:, N : 2 * N], i_sb[:], mi_sb[:, 0:N], ALU.mult
                    )
                else:
                    nc.vector.tensor_tensor(
                        mi_sb[:, N : 2 * N], i_sb[:], mi_sb[:, 0:N], ALU.mult
                    )
                nc.sync.dma_start(mi_d[t, :, :], mi_sb[:])

    nc.compile()
    return nc


def make_aux_inputs(W1, b1, W2, b2):
    W1 = np.asarray(W1)
    w1a = np.ascontiguousarray(W1[:D]).astype(HID_NP)
    w1b = np.ascontiguousarray(W1[D:]).astype(HID_NP)
    b1t = np.ascontiguousarray(np.tile(np.asarray(b1, np.float32), 4).reshape(128, 1))
    # col m carries W2 at partition-block (m % 4): every PSUM output row of the
    # reduce matmul is then a valid (replicated) logits row
    wd = np.zeros((128, 32), HID_NP)
    for m in range(32):
        gp = m % 4
        wd[32 * gp : 32 * gp + 32, m] = np.asarray(W2)
    b2t = np.full((128, 1), np.asarray(b2, np.float32)[0], np.float32)
    perm = np.zeros((8, 128, 128), np.float32)
    for h in range(2):
        for c in range(4):
            for q in range(4):
                for m in range(4):
                    perm[4 * h + c, 32 * q + m, 32 * m + 16 * h + 4 * q + c] = 1.0
    perm = np.ascontiguousarray(np.concatenate(list(perm), axis=1)).astype(HID_NP)
    return {
        "perm": perm,
        "w1a": w1a,
        "w1b": w1b,
        "b1t": b1t,
        "wd": wd,
        "b2t": b2t,
    }


TRACE = False
LAST_RESULTS = None


@functools.lru_cache(maxsize=1)
def _built_nc():
    return _build(T)


def kernel(**inputs):
    from concourse.bass_utils import run_bass_kernel_spmd

    h = np.asarray(inputs["h"])
    # (B, T, N, D) -> (B, D, T*N) so one DMA per core loads all hT with 8KB runs
    ht = np.ascontiguousarray(np.transpose(h, (0, 3, 1, 2)).reshape(B, D, -1)).astype(
        HID_NP
    )
    ifull = np.ascontiguousarray(np.asarray(inputs["I_full"], np.float32))
    aux = make_aux_inputs(inputs["W1"], inputs["b1"], inputs["W2"], inputs["b2"])

    nc = _built_nc()
    in_maps = [{"ht": ht[c], "ifull": ifull[c], **aux} for c in range(NCORES)]
    res = run_bass_kernel_spmd(
        nc, in_maps, core_ids=list(range(NCORES)), trace=TRACE
    )
    global LAST_RESULTS
    LAST_RESULTS = res
    mi = np.stack([res.results[c]["mi"] for c in range(NCORES)])
    return np.ascontiguousarray(mi[..., N:]), np.ascontiguousarray(mi[..., :N])


# revision 30
# speedup vs baseline: 18094.5924x; 1.0245x over previous
"""Trainium2 Bass kernel for the EdgeMask problem.

Computes, for h (B,T,N,d), I_full (B,T,N,N), MLP params W1 (2d,hid) b1 (hid,)
W2 (hid,) b2 (1,):
    li = h @ W1[:d]; lj = h @ W1[d:]
    hid = relu(li[:,:,:,None,:] + lj[:,:,None,:,:] + b1)
    M = sigmoid(hid @ W2 + b2);  I_sparse = I_full * M
Returns (I_sparse, M).

Sharding: data-parallel over B across 8 NeuronCores (B=8), no collectives.

Per-core layout (per (t) slice, N=128 nodes, d=128, K=32 hidden):
  - hT = h[t].T via PE transpose (d on partitions)
  - liT/ljT via col-tiled PE matmuls with W1a/W1b as stationary operands.
    Partition stacking p = 32*gp + k (4 replicas of the 32 hidden units).
    "Group" g covers rows i in {g, g+32, g+64, g+96} (i = g + 32*gp).
      R[32gp+k, j]  = lj[j,k] + b1[k]        (replicated 4x, ACT adds b1)
      S[32gp+k, g]  = li[g+32gp, k]          (li "stack", fp32)
  - Pointwise (the N^2*K hot loop): for each group g one fused op
      hid_g = max(R + S[:,g], 0)   -- DVE tensor_scalar(add,max) / ACT Relu+bias
  - Reduce over k via PE: lhsT = blockdiag(W2 x4) (128,4), col-tiled 4-way,
    rhs = 4 groups' hid packed into (128,512):
      logits'[i=4w+c+32m, j] at PSUM[32q+m, 128c+j]  (w = 4*half + q)
  - Compact 2 PSUM banks -> dense (128,128) via DMA, sigmoid(+b2) on ACT,
    I_full * M on DVE, DMA out.
"""

import functools

import numpy as np

import concourse.bass as bass
import concourse.mybir as mybir
import concourse.tile as tile
from concourse import bacc

F32 = mybir.dt.float32
F16 = mybir.dt.float16

B = 8
T = 32
N = 128
D = 128
K = 32  # hidden
NCORES = 8

AFT = mybir.ActivationFunctionType
ALU = mybir.AluOpType

# dtype of the hid (pointwise+reduce) path: F16 -> DVE 4x mode, F32 exact
HID_DT = F16
HID_NP = np.float16 if HID_DT == F16 else np.float32

# pointwise split: first ACT_SHARE_G groups on ACT, last POOL_SHARE_G on
# GPSIMD, rest on DVE
ACT_SHARE_G = 3
POOL_SHARE_G = 10
HID_BUFS = 24
SIG_DENSE = False
PW_SPREAD = False
MASK_ON_POOL = False
IO_BUFS = 4
OUT_BUFS = 3
RS_BUFS = 3
LILJ_BUFS = 2
MP_BUFS = 2
S_ON_ACT = False
MEXIT_SPLIT = False


def _pw_engine(g):
    if PW_SPREAD:
        # interleave: pool every 3rd, act sprinkled, rest dve
        w, c = divmod(g, 4)
        if c == 3 and w >= 8 - POOL_SHARE_G // 4 * 4:
            pass
        seq = (["dve"] * (K - ACT_SHARE_G - POOL_SHARE_G)
               + ["pool"] * POOL_SHARE_G + ["act"] * ACT_SHARE_G)
        # round-robin-ish deterministic shuffle
        return seq[(g * 7) % K]
    if g < ACT_SHARE_G:
        return "act"
    if g >= K - POOL_SHARE_G:
        return "pool"
    return "dve"


def _build(t_slices: int = T, skip=()):
    nc = bacc.Bacc(
        "TRN2", target_bir_lowering=False, debug=False, num_devices=NCORES
    )

    ht_d = nc.dram_tensor("ht", [D, t_slices * N], HID_DT, kind="ExternalInput")
    i_d = nc.dram_tensor("ifull", [t_slices, N, N], F32, kind="ExternalInput")
    w1a_d = nc.dram_tensor("w1a", [D, K], HID_DT, kind="ExternalInput")
    w1b_d = nc.dram_tensor("w1b", [D, K], HID_DT, kind="ExternalInput")
    b1t_d = nc.dram_tensor("b1t", [128, 1], F32, kind="ExternalInput")
    wd_d = nc.dram_tensor("wd", [128, 32], HID_DT, kind="ExternalInput")
    b2t_d = nc.dram_tensor("b2t", [128, 1], F32, kind="ExternalInput")
    perm_d = nc.dram_tensor("perm", [128, 8 * 128], HID_DT, kind="ExternalInput")

    # merged output: [..., 0:N] = M, [..., N:2N] = I_sparse (one store per slice)
    mi_d = nc.dram_tensor("mi", [t_slices, N, 2 * N], F32, kind="ExternalOutput")

    with tile.TileContext(nc) as tc:
        with (
            tc.tile_pool(name="const", bufs=1) as cpool,
            tc.tile_pool(name="hin", bufs=4) as hpool,
            tc.tile_pool(name="hts", bufs=3) as htpool,
            tc.tile_pool(name="rs", bufs=RS_BUFS) as rspool,
            tc.tile_pool(name="hid", bufs=HID_BUFS) as hidpool,
            tc.tile_pool(name="io", bufs=IO_BUFS) as iopool,
            tc.tile_pool(name="outp", bufs=OUT_BUFS) as opool,
            tc.tile_pool(name="psum", bufs=2, space="PSUM") as ppool,
        ):
            w1a_sb = cpool.tile([D, K], HID_DT)
            nc.sync.dma_start(w1a_sb[:], w1a_d[:])
            w1b_sb = cpool.tile([D, K], HID_DT)
            nc.sync.dma_start(w1b_sb[:], w1b_d[:])
            b1t_sb = cpool.tile([128, 1], F32)
            nc.sync.dma_start(b1t_sb[:], b1t_d[:])
            wd_sb = cpool.tile([128, 32], HID_DT)
            nc.sync.dma_start(wd_sb[:], wd_d[:])
            b2t_sb = cpool.tile([128, 1], F32)
            nc.sync.dma_start(b2t_sb[:], b2t_d[:])
            perm_sb = cpool.tile([128, 8 * 128], HID_DT)
            nc.sync.dma_start(perm_sb[:], perm_d[:])
            # all slices' hT, chunked so slice 0 can start early
            htall_sb = cpool.tile([D, t_slices * N], HID_DT)
            n_chunks = min(8, t_slices)
            chunk = t_slices * N // n_chunks
            for ci in range(n_chunks):
                nc.sync.dma_start(
                    htall_sb[:, ci * chunk : (ci + 1) * chunk],
                    ht_d[:, ci * chunk : (ci + 1) * chunk],
                )

            for t in range(t_slices):
                ht_sb = htall_sb[:, t * N : (t + 1) * N]

                # ---- liT / ljT, col-tiled (4 concurrent 32-col groups) ----
                lilj_ps = ppool.tile([128, N + K], F32, tag="lilj", bufs=LILJ_BUFS)
                for gp in range(4):
                    # ljT replicated: out[32gp+k, j] = lj[j, k]
                    nc.tensor.matmul(
                        lilj_ps[32 * gp : 32 * gp + 32, 0:N],
                        w1b_sb[:],
                        ht_sb,
                        tile_position=(0, 32 * gp),
                        skip_group_check=True,
                    )
                for gp in range(4):
                    # li stack: out[32gp+k, g] = li[g+32gp, k]
                    nc.tensor.matmul(
                        lilj_ps[32 * gp : 32 * gp + 32, N : N + K],
                        w1a_sb[:],
                        ht_sb[:, 32 * gp : 32 * gp + 32],
                        tile_position=(0, 32 * gp),
                        skip_group_check=True,
                    )

                # R = ljT_rep + b1 (cast to HID_DT); S = li stack (fp32)
                r_sb = rspool.tile([128, N], HID_DT, tag="r")
                nc.scalar.activation(
                    r_sb[:], lilj_ps[:, 0:N], AFT.Identity, bias=b1t_sb[:, 0:1]
                )
                s_sb = rspool.tile([128, K], F32, tag="s")
                if S_ON_ACT:
                    nc.scalar.copy(s_sb[:], lilj_ps[:, N : N + K])
                else:
                    nc.vector.tensor_copy(s_sb[:], lilj_ps[:, N : N + K])

                # ---- pointwise: hid_g = relu(R + S[:, g]) ----
                hbufs = [
                    hidpool.tile([128, 4 * N], HID_DT, tag="hid", name=f"hb{w}")
                    for w in range(8)
                ]
                for g in range(K):
                    w, c = divmod(g, 4)
                    dst = hbufs[w][:, c * N : (c + 1) * N]
                    eng = _pw_engine(g)
                    if eng == "act":
                        nc.scalar.activation(
                            dst, r_sb[:], AFT.Relu, bias=s_sb[:, g : g + 1]
                        )
                    elif eng == "pool":
                        nc.gpsimd.tensor_scalar(
                            dst, r_sb[:], s_sb[:, g : g + 1], 0.0, ALU.add, ALU.max
                        )
                    else:
                        nc.vector.tensor_scalar(
                            dst, r_sb[:], s_sb[:, g : g + 1], 0.0, ALU.add, ALU.max
                        )

                # ---- reduce over k on PE (col-tiled, 2 waves of 4) ----
                l_ps = [
                    ppool.tile([128, 4 * N], F32, tag="l0", name="l0"),
                    ppool.tile([128, 4 * N], F32, tag="l1", name="l1"),
                ]
                for w in range(8):
                    half, q = divmod(w, 4)
                    nc.tensor.matmul(
                        l_ps[half][32 * q : 32 * q + 32, :],
                        wd_sb[:],
                        hbufs[w][:],
                        tile_position=(0, 32 * q),
                    )

                # ---- PSUM exits (cast fp16) to SBUF ----
                # SIG_DENSE: raw-logit copies here, sigmoid after the permute.
                # else: sigmoid(+b2) applied here (sparse), permute carries M.
                lsp = [
                    opool.tile([128, 4 * N], HID_DT, tag="lsp0", name="lsp0"),
                    opool.tile([128, 4 * N], HID_DT, tag="lsp1", name="lsp1"),
                ]
                if SIG_DENSE:
                    nc.scalar.copy(lsp[0][:], l_ps[0][:])
                    nc.vector.tensor_copy(lsp[1][:], l_ps[1][:])
                else:
                    for half in range(2):
                        nc.scalar.activation(
                            lsp[half][:], l_ps[half][:], AFT.Sigmoid,
                            bias=b2t_sb[:, 0:1],
                        )

                # ---- un-permute logits on PE: 8 accumulating matmuls with 0/1
                # permutation matrices; P_b[32q+m, 32m+16h+4q+c] = 1, b=4h+c
                mp_ps = ppool.tile([128, N], F32, tag="mp", bufs=MP_BUFS)
                for b in range(8):
                    h, c = divmod(b, 4)
                    nc.tensor.matmul(
                        mp_ps[:],
                        perm_sb[:, 128 * b : 128 * (b + 1)],
                        lsp[h][:, 128 * c : 128 * (c + 1)],
                        start=(b == 0),
                        stop=(b == 7),
                    )
                # dense-PSUM exit; M lands in mi[:, 0:N]
                mi_sb = opool.tile([128, 2 * N], F32, tag="mi")
                if SIG_DENSE:
                    nc.scalar.activation(
                        mi_sb[:, 0:N], mp_ps[:], AFT.Sigmoid, bias=b2t_sb[:, 0:1]
                    )
                elif MEXIT_SPLIT:
                    nc.vector.tensor_copy(mi_sb[:, 0 : N // 2], mp_ps[:, 0 : N // 2])
                    nc.scalar.copy(mi_sb[:, N // 2 : N], mp_ps[:, N // 2 : N])
                else:
                    nc.vector.tensor_copy(mi_sb[:, 0:N], mp_ps[:])
                i_sb = iopool.tile([N, N], F32, tag="i")
                nc.sync.dma_start(i_sb[:], i_d[t, :, :])
                if MASK_ON_POOL:
                    nc.gpsimd.tensor_tensor(
                        mi_sb[:, N : 2 * N], i_sb[:], mi_sb[:, 0:N], ALU.mult
                    )
                else:
                    nc.vector.tensor_tensor(
                        mi_sb[:, N : 2 * N], i_sb[:], mi_sb[:, 0:N], ALU.mult
                    )
                nc.sync.dma_start(mi_d[t, :, :], mi_sb[:])

    nc.compile()
    return nc


def make_aux_inputs(W1, b1, W2, b2):
    W1 = np.asarray(W1)
    w1a = np.ascontiguousarray(W1[:D]).astype(HID_NP)
    w1b = np.ascontiguousarray(W1[D:]).astype(HID_NP)
    b1t = np.ascontiguousarray(np.tile(np.asarray(b1, np.float32), 4).reshape(128, 1))
    # col m carries W2 at partition-block (m % 4): every PSUM output row of the
    # reduce matmul is then a valid (replicated) logits row
    wd = np.zeros((128, 32), HID_NP)
    for m in range(32):
        gp = m % 4
        wd[32 * gp : 32 * gp + 32, m] = np.asarray(W2)
    b2t = np.full((128, 1), np.asarray(b2, np.float32)[0], np.float32)
    perm = np.zeros((8, 128, 128), np.float32)
    for h in range(2):
        for c in range(4):
            for q in range(4):
                for m in range(4):
                    perm[4 * h + c, 32 * q + m, 32 * m + 16 * h + 4 * q + c] = 1.0
    perm = np.ascontiguousarray(np.concatenate(list(perm), axis=1)).astype(HID_NP)
    return {
        "perm": perm,
        "w1a": w1a,
        "w1b": w1b,
        "b1t": b1t,
        "wd": wd,
        "b2t": b2t,
    }


TRACE = False
LAST_RESULTS = None


@functools.lru_cache(maxsize=1)
def _built_nc():
    return _build(T)


def kernel(**inputs):
    from concourse.bass_utils import run_bass_kernel_spmd

    h = np.asarray(inputs["h"])
    # (B, T, N, D) -> (B, D, T*N) so one DMA per core loads all hT with 8KB runs
    ht = np.ascontiguousarray(np.transpose(h, (0, 3, 1, 2)).reshape(B, D, -1)).astype(
        HID_NP
    )
    ifull = np.ascontiguousarray(np.asarray(inputs["I_full"], np.float32))
    aux = make_aux_inputs(inputs["W1"], inputs["b1"], inputs["W2"], inputs["b2"])

    nc = _built_nc()
    in_maps = [{"ht": ht[c], "ifull": ifull[c], **aux} for c in range(NCORES)]
    res = run_bass_kernel_spmd(
        nc, in_maps, core_ids=list(range(NCORES)), trace=TRACE
    )
    global LAST_RESULTS
    LAST_RESULTS = res
    mi = np.stack([res.results[c]["mi"] for c in range(NCORES)])
    return np.ascontiguousarray(mi[..., N:]), np.ascontiguousarray(mi[..., :N])
